# revision 27
# baseline (speedup 1.0000x reference)
"""MixtureOfSoftMaxACF Trainium2 kernel (v6).

Per-core (data-parallel over BS=8 across 8 cores, batch b per core):
  qt[b] memory reinterpreted as QQ[2, 2048, 64] (contiguous halves), same kt.
  For m in {0,1}:  S_m = QQ[m] @ KK[m].T / sqrt(128);  P_m = softmax(S_m, axis=-1)
  out[b] = (p0 * P_0 + p1 * P_1) @ vt[b]
  p: mixture prior (softmax over batch axis) -> computed on host, passed per-core.

Design notes (the ScalarE exp stream, 64 x [128,1024] ~ 73us busy, is the
floor; everything else is arranged to hide under it and keep the PE HAM
clock-gate at 8/8):
  - Staging is bf16-first (cast the staged fp32 per 4-chunk group, then bf16
    PE transposes) and split: only what the first score pairs need runs
    before the main loop; the rest is deferred work fed into the instruction
    stream between pairs, in an order proven against its consumers.
  - Half-swapped copies qt_sw/kt_sw (rows 64-127 <-> 0-63, SBUF-SBUF DMA on
    the gpsimd queue so their dependency waits don't block the sync DMA
    queue) let any (mixture, chunk-parity) land on the row group the score
    pairing wants; chunk pairs are emitted interleaved so consecutive queue
    entries alternate PE row groups and run concurrently.
  - E stays bf16 (fp8 E puts the softmax-weight tail error at 2.3e-2 > 2e-2).
  - Denominator over E2[j] = E[2j]+E[2j+1] (DVE pairwise sums, half the PE
    stream), deferred one phase and fed between pairs as gap filler; the
    last phase computes its own inline (h0 through the ps_r bank from pair 0,
    h1 through ps_d once the deferred backlog drains) to keep the tail short.
  - bf16 evacuation/normalize/combine; bf16 output transposes.
  - PSUM: scores 2x2 + outT 2 + Drep 1 + shared(D2/res) 1 = 8 banks.
"""

import math
from contextlib import ExitStack

import numpy as np

import concourse.bass as bass
import concourse.bacc as bacc
import concourse.mybir as mybir
import concourse.tile as tile
from concourse.bass_utils import run_bass_kernel_spmd
from concourse.masks import make_identity

BS = 8
N = 2048          # queries
NK = 2048         # keys
DK = 128
M = 2
D = DK // M       # 64
DV = 128
TEMP = math.sqrt(DK)
NCH = NK // 128   # 16 key chunks
NPAIR = NCH // 2  # 8
QH = 2            # query halves
QHN = N // QH     # 1024
HG = NCH // 4     # 4 chunks per staging group

F32 = mybir.dt.float32
BF16 = mybir.dt.bfloat16

_NC = None
LAST_RESULT = None  # BassKernelResults of last run (test.py reads this)


def _build():
    nc = bacc.Bacc(None)
    qt_d = nc.declare_dram_parameter("qt_b", [N, DK], F32, isOutput=False)
    kt_d = nc.declare_dram_parameter("kt_b", [NK, DK], F32, isOutput=False)
    vt_d = nc.declare_dram_parameter("vt_b", [NK, DK], F32, isOutput=False)
    pr_d = nc.declare_dram_parameter("pr_b", [1, M], F32, isOutput=False)
    out_d = nc.declare_dram_parameter("out_b", [N, DK], F32, isOutput=True)

    with ExitStack() as ctx:
        tc = ctx.enter_context(tile.TileContext(nc))
        sb = ctx.enter_context(tc.tile_pool(name="sb", bufs=1))
        ps = ctx.enter_context(tc.tile_pool(name="ps", bufs=1, space="PSUM"))
        const = sbig = epool = e2pool = npool = sb
        ps_s = ps_acc = ps_d = ps_r = ps

        # ---- constants ----
        ident_f = const.tile([128, 128], F32)
        make_identity(nc, ident_f)
        ident_b = const.tile([128, 128], BF16)
        nc.vector.tensor_copy(ident_b, ident_f)
        ones_f = const.tile([128, 128], F32)
        nc.vector.memset(ones_f, 1.0)
        ones_w = const.tile([128, 128], BF16)
        nc.vector.tensor_copy(ones_w, ones_f)
        pr_sb = const.tile([128, M], F32)

        # ---- input staging ----
        # stage[p, c, m*64+d] = src[m*N*D + (c*128+p)*64 + d]; the bf16
        # [128,128] chunk-transpose then has mixture m's d-rows at partitions
        # m*64..m*64+63.  Sync DMA queue carries the DRAM loads in
        # most-needed-first order; gpsimd carries the SBUF-SBUF half-swaps
        # (their dependency waits must not block the sync queue).
        stage_q = sbig.tile([128, NCH, DK], F32, tag="stage_q")
        stage_k = sbig.tile([128, NCH, DK], F32, tag="stage_k")
        stage_qb = sbig.tile([128, NCH, DK], BF16, tag="stage_qb")
        stage_kb = sbig.tile([128, NCH, DK], BF16, tag="stage_kb")
        v_st = sbig.tile([128, NCH, DV], F32)
        v_sb = sbig.tile([128, NCH, DV], BF16)
        qt_t = sbig.tile([128, N], BF16)
        kt_t = sbig.tile([128, NK], BF16)
        qt_sw = sbig.tile([128, N], BF16)
        kt_sw = sbig.tile([128, NK], BF16)

        def stage_piece(dst, src, m, h, eng=None):
            (eng or nc.sync).dma_start(
                out=dst[:, h * HG:(h + 1) * HG, m * D:(m + 1) * D],
                in_=bass.AP(tensor=src, offset=m * N * D + h * HG * 128 * D,
                            ap=[[D, 128], [128 * D, HG], [1, D]]))

        def v_piece(h, eng=None):
            (eng or nc.sync).dma_start(
                out=v_st[:, h * HG:(h + 1) * HG, :],
                in_=bass.AP(tensor=vt_d, offset=h * HG * 128 * DK,
                            ap=[[DK, 128], [128 * DK, HG], [1, DV]]))

        def group_cast(stf, stb, g):
            nc.vector.tensor_copy(stb[:, g * HG:(g + 1) * HG, :],
                                  stf[:, g * HG:(g + 1) * HG, :])

        def v_cast(g):
            nc.vector.tensor_copy(v_sb[:, g * HG:(g + 1) * HG, :],
                                  v_st[:, g * HG:(g + 1) * HG, :])

        def tp_chunk(stb, dst, c):
            tp = ps_s.tile([128, 128], BF16, tag="s", bufs=2)
            nc.tensor.transpose(tp, stb[:, c, :], ident_b)
            nc.vector.tensor_copy(dst[:, c * 128:(c + 1) * 128], tp)

        def swap_group(src_t, dst_t, g):
            sl = slice(g * HG * 128, (g + 1) * HG * 128)
            for half in range(2):
                nc.gpsimd.dma_start(
                    out=dst_t[64 * (1 - half):64 * (2 - half), sl],
                    in_=src_t[64 * half:64 * half + 64, sl])

        # staging DMAs split across the two issue queues, most-needed-first
        # per queue; pre-loop swaps go at the END of the gpsimd section so
        # their dependency waits don't block later loads on that queue.
        for m in range(M):
            stage_piece(stage_k, kt_d, m, 0)
        for m in range(M):
            stage_piece(stage_q, qt_d, m, 0, nc.gpsimd)
        for m in range(M):
            stage_piece(stage_q, qt_d, m, 1)
        v_piece(0, nc.gpsimd)
        for h in range(1, 4):
            for m in range(M):
                stage_piece(stage_k, kt_d, m, h)
        v_piece(1)
        nc.sync.dma_start(out=pr_sb,
                          in_=bass.AP(tensor=pr_d, offset=0,
                                      ap=[[0, 128], [1, M]]))
        for h in range(2, 4):
            for m in range(M):
                stage_piece(stage_q, qt_d, m, h)
        v_piece(2)
        v_piece(3)

        # pre-loop staging: exactly what score pairs 0-1 and AV group 0 need
        group_cast(stage_k, stage_kb, 0)
        for c in range(HG):
            tp_chunk(stage_kb, kt_t, c)
        swap_group(kt_t, kt_sw, 0)
        for g in range(2):
            group_cast(stage_q, stage_qb, g)
            for c in range(g * HG, (g + 1) * HG):
                tp_chunk(stage_qb, qt_t, c)
            swap_group(qt_t, qt_sw, g)
        v_cast(0)

        # deferred staging, ordered against its consumers (popped 3/pair):
        # kt group g must be transposed+swapped before score pair 2g emits,
        # v group g cast before AV chunk 4g.
        pend = []

        def kt_item(c):
            def it():
                if c % HG == 0:
                    group_cast(stage_k, stage_kb, c // HG)
                tp_chunk(stage_kb, kt_t, c)
            return it

        def qt_item(c):
            def it():
                if c % HG == 0:
                    group_cast(stage_q, stage_qb, c // HG)
                tp_chunk(stage_qb, qt_t, c)
            return it

        for g in range(1, 4):
            for c in range(g * HG, (g + 1) * HG):
                pend.append(kt_item(c))
            pend.append(lambda g=g: swap_group(kt_t, kt_sw, g))
            pend.append(lambda g=g: v_cast(g))
        late_qt = []
        for g in range(2, 4):
            for c in range(g * HG, (g + 1) * HG):
                late_qt.append(qt_item(c))
            late_qt.append(lambda g=g: swap_group(qt_t, qt_sw, g))

        # ---- attention ----
        scale = 1.0 / TEMP
        all_res = {}  # (qh, m) -> [oT_bf, [drec_h0, drec_h1]]

        def pop_pend(k):
            for _ in range(k):
                if pend:
                    pend.pop(0)()

        NOCT = NPAIR // 4

        def denom_mm(Drep, E8, j, hf, start, stop):
            sl = slice(hf * 512, (hf + 1) * 512)
            nc.tensor.matmul(Drep, lhsT=ones_w, rhs=E8[:, j, sl],
                             start=start, stop=stop)

        def make_denom_pend(E8, key):
            items = []
            res = all_res[key]
            for hf in range(2):
                Drep = ps_d.tile([128, 512], F32, tag="D")
                for j in range(NOCT):
                    items.append(lambda j=j, hf=hf, Drep=Drep: denom_mm(
                        Drep, E8, j, hf, j == 0, j == NOCT - 1))

                def recip(hf=hf, Drep=Drep):
                    dre = npool.tile([128, 512], F32, tag="drec", bufs=6)
                    nc.vector.reciprocal_approx_fast(dre, Drep)
                    res[1][hf] = dre

                items.append(recip)
            return items

        def emit_combine_half(qh, hf):
            # rT2 = p0*oT0*drec0 + p1*oT1*drec1 in the [dv, q] domain, then
            # bf16 PE transpose back to [q, dv] through the shared ps_r bank.
            oT0, dr0 = all_res[(qh, 0)]
            oT1, dr1 = all_res[(qh, 1)]
            sl = slice(hf * 512, (hf + 1) * 512)
            otn0 = npool.tile([128, 512], BF16, tag="otn0", bufs=2)
            nc.vector.tensor_mul(otn0, oT0[:, sl], dr0[hf])
            otn1 = npool.tile([128, 512], BF16, tag="otn1", bufs=2)
            nc.vector.tensor_mul(otn1, oT1[:, sl], dr1[hf])
            rT2 = npool.tile([128, 512], BF16, tag="rT2", bufs=2)
            nc.vector.tensor_add(rT2, otn0, otn1)
            res_f = ps_r.tile([128, 512], F32, tag="rd2")
            res_b = res_f.bitcast(BF16)[:, 0:512]
            res_sb = npool.tile([128, 512], F32, tag="res_sb", bufs=2)
            for tt in range(4):
                nc.tensor.transpose(res_b[:, tt * 128:(tt + 1) * 128],
                                    rT2[:, tt * 128:(tt + 1) * 128], ident_b)
            nc.vector.tensor_copy(res_sb, res_b)
            nc.sync.dma_start(
                out=bass.AP(tensor=out_d, offset=(qh * QHN + hf * 512) * DK,
                            ap=[[DK, 128], [128 * DK, 4], [1, DV]]),
                in_=res_sb.rearrange("p (t d) -> p t d", d=DV),
            )

        phases = [(qh, m) for qh in range(QH) for m in range(M)]
        state = {}

        def ensure_state(key):
            if key not in state:
                outT = ps_acc.tile([128, QHN], F32, tag="outT")
                E = epool.tile([128, NCH, QHN], BF16, tag="E")
                E2 = e2pool.tile([128, NPAIR, QHN], BF16, tag="E2", bufs=2)
                E4 = e2pool.tile([128, NPAIR // 2, QHN], BF16, tag="E4", bufs=2)
                E8 = e2pool.tile([128, NPAIR // 4, QHN], BF16, tag="E8", bufs=2)
                state[key] = dict(outT=outT, E=E, E2=E2, E4=E4, E8=E8)
                all_res[key] = [None, [None, None]]
            return state[key]

        def emit_chunk_scores(key, c):
            # one chunk's two score halves; chunk parity picks the PE row
            # group (via the half-swapped copies) so consecutive chunks'
            # matmuls can overlap on opposite row groups.
            qh, m = key
            p = c % 2
            nat = (p == 0) == (m == 0)
            kt_src = kt_t if nat else kt_sw
            qt_src = qt_t if nat else qt_sw
            ro = 64 * p
            s = ps_s.tile([128, QHN], F32, tag="s", bufs=2)
            for hf in range(2):
                sl = slice(hf * 512, (hf + 1) * 512)
                nc.tensor.matmul(
                    s[:, sl],
                    lhsT=kt_src[ro:ro + 64, c * 128:(c + 1) * 128],
                    rhs=qt_src[ro:ro + 64,
                               qh * QHN + hf * 512: qh * QHN + (hf + 1) * 512],
                    start=True, stop=True,
                )
            return s

        def emit_av(st, c):
            outT, E = st["outT"], st["E"]
            for hf in range(2):
                sl = slice(hf * 512, (hf + 1) * 512)
                nc.tensor.matmul(outT[:, sl], lhsT=v_sb[:, c, :],
                                 rhs=E[:, c, sl],
                                 start=(c == 0), stop=(c == NCH - 1))

        last_key = phases[-1]
        # chunk-granular software pipeline with a 2-chunk score lookahead
        # (the depth of the score-tile ring): right after exp(c) is emitted,
        # emit exactly the PE work that its completion unblocks -- the
        # scores of c+2 (tile WAR) and AV of c -- so the matmul queue head
        # is never parked on a later exp.
        flat = [(key, c) for key in phases for c in range(NCH)]
        sq = []  # pending score tiles, per flat index
        for i, key in enumerate(phases):
            qh, m = key
            st = ensure_state(key)
            if i == 0:
                sq.append(emit_chunk_scores(key, 0))
                sq.append(emit_chunk_scores(key, 1))
            inline_d = key == last_key
            if inline_d:
                Drep_h0 = ps_r.tile([128, 512], F32, tag="rd2")
                Drep_h1 = None
            for c in range(NCH):
                cur_s = sq.pop(0)
                nc.scalar.activation(st["E"][:, c, :], cur_s,
                                     mybir.ActivationFunctionType.Exp,
                                     scale=scale)
                # look ahead 2 chunks (possibly into the next phase)
                fi = i * NCH + c + 2
                if fi < len(flat):
                    nkey, nck = flat[fi]
                    ensure_state(nkey)
                    sq.append(emit_chunk_scores(nkey, nck))
                emit_av(st, c)
                pop_pend(2)
                if c == NCH - 1 and not inline_d:
                    # evacuate outT (bf16, prior-scaled) ahead of the final
                    # tree adds so the accumulator bank rotates sooner
                    ot = npool.tile([128, QHN], BF16, tag="oT", bufs=3)
                    nc.vector.tensor_scalar_mul(ot, st["outT"],
                                                pr_sb[:, m:m + 1])
                    all_res[key][0] = ot
                if c % 2 == 1:
                    j = c // 2
                    nc.vector.tensor_add(st["E2"][:, j, :],
                                         st["E"][:, c - 1, :],
                                         st["E"][:, c, :])
                    if j % 2 == 1:
                        jq = j // 2
                        nc.vector.tensor_add(st["E4"][:, jq, :],
                                             st["E2"][:, j - 1, :],
                                             st["E2"][:, j, :])
                        if jq % 2 == 1:
                            jo = jq // 2
                            nc.vector.tensor_add(st["E8"][:, jo, :],
                                                 st["E4"][:, jq - 1, :],
                                                 st["E4"][:, jq, :])
                            if inline_d:
                                if jo == 0:
                                    Drep_h1 = ps_d.tile([128, 512], F32,
                                                        tag="D")
                                denom_mm(Drep_h0, st["E8"], jo, 0, jo == 0,
                                         jo == NOCT - 1)
                                denom_mm(Drep_h1, st["E8"], jo, 1, jo == 0,
                                         jo == NOCT - 1)
            if inline_d:
                ot = npool.tile([128, QHN], BF16, tag="oT", bufs=3)
                nc.vector.tensor_scalar_mul(ot, st["outT"], pr_sb[:, m:m + 1])
                all_res[key][0] = ot
            if inline_d:
                for hf, Drep in ((0, Drep_h0), (1, Drep_h1)):
                    dre = npool.tile([128, 512], F32, tag="drec", bufs=6)
                    nc.vector.reciprocal_approx_fast(dre, Drep)
                    all_res[key][1][hf] = dre
            else:
                if key == (0, 0):
                    # remaining qt staging must precede the denominator
                    # backlog: phase (1,0)'s first scores (emitted near the
                    # end of (0,1)) need qt groups 2-3 transposed+swapped.
                    pend.extend(late_qt)
                pend.extend(make_denom_pend(st["E8"], key))
            if m == M - 1:
                pend.append(lambda qh=qh: emit_combine_half(qh, 0))
                pend.append(lambda qh=qh: emit_combine_half(qh, 1))
            del state[key]
        pop_pend(len(pend))
    return nc


def _get_nc():
    global _NC
    if _NC is None:
        _NC = _build()
        _NC.finalize()  # Bacc.compile(): event sems, reg alloc, wait legalization
    return _NC


def _prior(qt, kernel):
    bar_qt = qt.astype(np.float32).mean(axis=1)          # (BS, dk)
    logits = kernel.astype(np.float32) @ bar_qt.T        # (m, BS)
    z = logits - logits.max(axis=1, keepdims=True)
    ez = np.exp(z)
    pm = ez / ez.sum(axis=1, keepdims=True)              # softmax over batch axis
    return pm.reshape(-1)


def kernel(qt, kt, vt, kernel):
    global LAST_RESULT
    import os
    nc = _get_nc()
    prior_flat = _prior(qt, kernel)
    in_maps = []
    for b in range(BS):
        pr = np.array([[prior_flat[2 * b], prior_flat[2 * b + 1]]], dtype=np.float32)
        in_maps.append({
            "qt_b": np.ascontiguousarray(qt[b], dtype=np.float32),
            "kt_b": np.ascontiguousarray(kt[b], dtype=np.float32),
            "vt_b": np.ascontiguousarray(vt[b], dtype=np.float32),
            "pr_b": pr,
        })
    trace = bool(int(os.environ.get("KERNEL_TRACE", "0")))
    res = run_bass_kernel_spmd(nc, in_maps, list(range(BS)), trace=trace)
    LAST_RESULT = res
    out = np.stack([np.asarray(res.results[b]["out_b"]).reshape(N, DK) for b in range(BS)])
    return out.astype(np.float32)


# revision 28
# speedup vs baseline: 1.0406x; 1.0406x over previous
"""MixtureOfSoftMaxACF Trainium2 kernel (v6).

Per-core (data-parallel over BS=8 across 8 cores, batch b per core):
  qt[b] memory reinterpreted as QQ[2, 2048, 64] (contiguous halves), same kt.
  For m in {0,1}:  S_m = QQ[m] @ KK[m].T / sqrt(128);  P_m = softmax(S_m, axis=-1)
  out[b] = (p0 * P_0 + p1 * P_1) @ vt[b]
  p: mixture prior (softmax over batch axis) -> computed on host, passed per-core.

Design notes (the ScalarE exp stream, 64 x [128,1024] ~ 73us busy, is the
floor; everything else is arranged to hide under it and keep the PE HAM
clock-gate at 8/8):
  - Staging is bf16-first (cast the staged fp32 per 4-chunk group, then bf16
    PE transposes) and split: only what the first score pairs need runs
    before the main loop; the rest is deferred work fed into the instruction
    stream between pairs, in an order proven against its consumers.
  - Half-swapped copies qt_sw/kt_sw (rows 64-127 <-> 0-63, SBUF-SBUF DMA on
    the gpsimd queue so their dependency waits don't block the sync DMA
    queue) let any (mixture, chunk-parity) land on the row group the score
    pairing wants; chunk pairs are emitted interleaved so consecutive queue
    entries alternate PE row groups and run concurrently.
  - E stays bf16 (fp8 E puts the softmax-weight tail error at 2.3e-2 > 2e-2).
  - Denominator over E2[j] = E[2j]+E[2j+1] (DVE pairwise sums, half the PE
    stream), deferred one phase and fed between pairs as gap filler; the
    last phase computes its own inline (h0 through the ps_r bank from pair 0,
    h1 through ps_d once the deferred backlog drains) to keep the tail short.
  - bf16 evacuation/normalize/combine; bf16 output transposes.
  - PSUM: scores 2x2 + outT 2 + Drep 1 + shared(D2/res) 1 = 8 banks.
"""

import math
from contextlib import ExitStack

import numpy as np

import concourse.bass as bass
import concourse.bacc as bacc
import concourse.mybir as mybir
import concourse.tile as tile
from concourse.bass_utils import run_bass_kernel_spmd
from concourse.masks import make_identity

BS = 8
N = 2048          # queries
NK = 2048         # keys
DK = 128
M = 2
D = DK // M       # 64
DV = 128
TEMP = math.sqrt(DK)
NCH = NK // 128   # 16 key chunks
NPAIR = NCH // 2  # 8
QH = 2            # query halves
QHN = N // QH     # 1024
HG = NCH // 4     # 4 chunks per staging group

F32 = mybir.dt.float32
BF16 = mybir.dt.bfloat16

_NC = None
LAST_RESULT = None  # BassKernelResults of last run (test.py reads this)


def _build():
    nc = bacc.Bacc(None)
    qt_d = nc.declare_dram_parameter("qt_b", [N, DK], F32, isOutput=False)
    kt_d = nc.declare_dram_parameter("kt_b", [NK, DK], F32, isOutput=False)
    vt_d = nc.declare_dram_parameter("vt_b", [NK, DK], F32, isOutput=False)
    pr_d = nc.declare_dram_parameter("pr_b", [1, M], F32, isOutput=False)
    out_d = nc.declare_dram_parameter("out_b", [N, DK], F32, isOutput=True)

    with ExitStack() as ctx:
        tc = ctx.enter_context(tile.TileContext(nc))
        sb = ctx.enter_context(tc.tile_pool(name="sb", bufs=1))
        ps = ctx.enter_context(tc.tile_pool(name="ps", bufs=1, space="PSUM"))
        const = sbig = epool = e2pool = npool = sb
        ps_s = ps_acc = ps_d = ps_r = ps

        # ---- constants ----
        ident_f = const.tile([128, 128], F32)
        make_identity(nc, ident_f)
        ident_b = const.tile([128, 128], BF16)
        nc.vector.tensor_copy(ident_b, ident_f)
        ones_f = const.tile([128, 128], F32)
        nc.vector.memset(ones_f, 1.0)
        ones_w = const.tile([128, 128], BF16)
        nc.vector.tensor_copy(ones_w, ones_f)
        pr_sb = const.tile([128, M], F32)

        # ---- input staging ----
        # stage[p, c, m*64+d] = src[m*N*D + (c*128+p)*64 + d]; the bf16
        # [128,128] chunk-transpose then has mixture m's d-rows at partitions
        # m*64..m*64+63.  Sync DMA queue carries the DRAM loads in
        # most-needed-first order; gpsimd carries the SBUF-SBUF half-swaps
        # (their dependency waits must not block the sync queue).
        stage_q = sbig.tile([128, NCH, DK], F32, tag="stage_q")
        stage_k = sbig.tile([128, NCH, DK], F32, tag="stage_k")
        stage_qb = sbig.tile([128, NCH, DK], BF16, tag="stage_qb")
        stage_kb = sbig.tile([128, NCH, DK], BF16, tag="stage_kb")
        v_st = sbig.tile([128, NCH, DV], F32)
        v_sb = sbig.tile([128, NCH, DV], BF16)
        qt_t = sbig.tile([128, N], BF16)
        kt_t = sbig.tile([128, NK], BF16)
        qt_sw = sbig.tile([128, N], BF16)
        kt_sw = sbig.tile([128, NK], BF16)

        def stage_piece(dst, src, m, h):
            nc.sync.dma_start(
                out=dst[:, h * HG:(h + 1) * HG, m * D:(m + 1) * D],
                in_=bass.AP(tensor=src, offset=m * N * D + h * HG * 128 * D,
                            ap=[[D, 128], [128 * D, HG], [1, D]]))

        def v_piece(h):
            nc.sync.dma_start(
                out=v_st[:, h * HG:(h + 1) * HG, :],
                in_=bass.AP(tensor=vt_d, offset=h * HG * 128 * DK,
                            ap=[[DK, 128], [128 * DK, HG], [1, DV]]))

        def group_cast(stf, stb, g):
            nc.vector.tensor_copy(stb[:, g * HG:(g + 1) * HG, :],
                                  stf[:, g * HG:(g + 1) * HG, :])

        def v_cast(g):
            nc.vector.tensor_copy(v_sb[:, g * HG:(g + 1) * HG, :],
                                  v_st[:, g * HG:(g + 1) * HG, :])

        def tp_chunk(stb, dst, c):
            tp = ps_s.tile([128, 128], BF16, tag="s", bufs=2)
            nc.tensor.transpose(tp, stb[:, c, :], ident_b)
            nc.vector.tensor_copy(dst[:, c * 128:(c + 1) * 128], tp)

        def swap_group(src_t, dst_t, g):
            sl = slice(g * HG * 128, (g + 1) * HG * 128)
            for half in range(2):
                nc.gpsimd.dma_start(
                    out=dst_t[64 * (1 - half):64 * (2 - half), sl],
                    in_=src_t[64 * half:64 * half + 64, sl])

        # sync-queue DMAs, most-needed-first
        for m in range(M):
            stage_piece(stage_k, kt_d, m, 0)
        for h in range(2):
            for m in range(M):
                stage_piece(stage_q, qt_d, m, h)
        v_piece(0)
        for h in range(1, 4):
            for m in range(M):
                stage_piece(stage_k, kt_d, m, h)
        v_piece(1)
        nc.sync.dma_start(out=pr_sb,
                          in_=bass.AP(tensor=pr_d, offset=0,
                                      ap=[[0, 128], [1, M]]))
        for h in range(2, 4):
            for m in range(M):
                stage_piece(stage_q, qt_d, m, h)
        v_piece(2)
        v_piece(3)

        # pre-loop staging: exactly what score pairs 0-1 and AV group 0 need
        group_cast(stage_k, stage_kb, 0)
        for c in range(HG):
            tp_chunk(stage_kb, kt_t, c)
        swap_group(kt_t, kt_sw, 0)
        for g in range(2):
            group_cast(stage_q, stage_qb, g)
            for c in range(g * HG, (g + 1) * HG):
                tp_chunk(stage_qb, qt_t, c)
            swap_group(qt_t, qt_sw, g)
        v_cast(0)

        # deferred staging, ordered against its consumers (popped 3/pair):
        # kt group g must be transposed+swapped before score pair 2g emits,
        # v group g cast before AV chunk 4g.
        pend = []

        def kt_item(c):
            def it():
                if c % HG == 0:
                    group_cast(stage_k, stage_kb, c // HG)
                tp_chunk(stage_kb, kt_t, c)
            return it

        def qt_item(c):
            def it():
                if c % HG == 0:
                    group_cast(stage_q, stage_qb, c // HG)
                tp_chunk(stage_qb, qt_t, c)
            return it

        for g in range(1, 4):
            for c in range(g * HG, (g + 1) * HG):
                pend.append(kt_item(c))
            pend.append(lambda g=g: swap_group(kt_t, kt_sw, g))
            pend.append(lambda g=g: v_cast(g))
        late_qt = []
        for g in range(2, 4):
            for c in range(g * HG, (g + 1) * HG):
                late_qt.append(qt_item(c))
            late_qt.append(lambda g=g: swap_group(qt_t, qt_sw, g))

        # ---- attention ----
        scale = 1.0 / TEMP
        all_res = {}  # (qh, m) -> [oT_bf, [drec_h0, drec_h1]]

        def pop_pend(k):
            for _ in range(k):
                if pend:
                    pend.pop(0)()

        NQUAD = NPAIR // 2

        def denom_mm(Drep, E4, j, hf, start, stop):
            sl = slice(hf * 512, (hf + 1) * 512)
            nc.tensor.matmul(Drep, lhsT=ones_w, rhs=E4[:, j, sl],
                             start=start, stop=stop)

        def make_denom_pend(E4, key):
            items = []
            res = all_res[key]
            for hf in range(2):
                Drep = ps_d.tile([128, 512], F32, tag="D")
                for j in range(NQUAD):
                    items.append(lambda j=j, hf=hf, Drep=Drep: denom_mm(
                        Drep, E4, j, hf, j == 0, j == NQUAD - 1))

                def recip(hf=hf, Drep=Drep):
                    dre = npool.tile([128, 512], F32, tag="drec", bufs=6)
                    nc.vector.reciprocal_approx_fast(dre, Drep)
                    res[1][hf] = dre

                items.append(recip)
            return items

        def emit_combine_half(qh, hf):
            # rT2 = p0*oT0*drec0 + p1*oT1*drec1 in the [dv, q] domain, then
            # bf16 PE transpose back to [q, dv] through the shared ps_r bank.
            oT0, dr0 = all_res[(qh, 0)]
            oT1, dr1 = all_res[(qh, 1)]
            sl = slice(hf * 512, (hf + 1) * 512)
            otn0 = npool.tile([128, 512], BF16, tag="otn0", bufs=2)
            nc.vector.tensor_mul(otn0, oT0[:, sl], dr0[hf])
            otn1 = npool.tile([128, 512], BF16, tag="otn1", bufs=2)
            nc.vector.tensor_mul(otn1, oT1[:, sl], dr1[hf])
            rT2 = npool.tile([128, 512], BF16, tag="rT2", bufs=2)
            nc.vector.tensor_add(rT2, otn0, otn1)
            res_f = ps_r.tile([128, 512], F32, tag="rd2")
            res_b = res_f.bitcast(BF16)[:, 0:512]
            res_sb = npool.tile([128, 512], F32, tag="res_sb", bufs=2)
            for tt in range(4):
                nc.tensor.transpose(res_b[:, tt * 128:(tt + 1) * 128],
                                    rT2[:, tt * 128:(tt + 1) * 128], ident_b)
            nc.vector.tensor_copy(res_sb, res_b)
            nc.sync.dma_start(
                out=bass.AP(tensor=out_d, offset=(qh * QHN + hf * 512) * DK,
                            ap=[[DK, 128], [128 * DK, 4], [1, DV]]),
                in_=res_sb.rearrange("p (t d) -> p t d", d=DV),
            )

        phases = [(qh, m) for qh in range(QH) for m in range(M)]
        state = {}

        def ensure_state(key):
            if key not in state:
                outT = ps_acc.tile([128, QHN], F32, tag="outT")
                E = epool.tile([128, NCH, QHN], BF16, tag="E")
                E2 = e2pool.tile([128, NPAIR, QHN], BF16, tag="E2", bufs=2)
                E4 = e2pool.tile([128, NPAIR // 2, QHN], BF16, tag="E4", bufs=2)
                state[key] = dict(outT=outT, E=E, E2=E2, E4=E4)
                all_res[key] = [None, [None, None]]
            return state[key]

        def emit_chunk_scores(key, c):
            # one chunk's two score halves; chunk parity picks the PE row
            # group (via the half-swapped copies) so consecutive chunks'
            # matmuls can overlap on opposite row groups.
            qh, m = key
            p = c % 2
            nat = (p == 0) == (m == 0)
            kt_src = kt_t if nat else kt_sw
            qt_src = qt_t if nat else qt_sw
            ro = 64 * p
            s = ps_s.tile([128, QHN], F32, tag="s", bufs=2)
            for hf in range(2):
                sl = slice(hf * 512, (hf + 1) * 512)
                nc.tensor.matmul(
                    s[:, sl],
                    lhsT=kt_src[ro:ro + 64, c * 128:(c + 1) * 128],
                    rhs=qt_src[ro:ro + 64,
                               qh * QHN + hf * 512: qh * QHN + (hf + 1) * 512],
                    start=True, stop=True,
                )
            return s

        def emit_av(st, c):
            outT, E = st["outT"], st["E"]
            for hf in range(2):
                sl = slice(hf * 512, (hf + 1) * 512)
                nc.tensor.matmul(outT[:, sl], lhsT=v_sb[:, c, :],
                                 rhs=E[:, c, sl],
                                 start=(c == 0), stop=(c == NCH - 1))

        last_key = phases[-1]
        # chunk-granular software pipeline with a 2-chunk score lookahead
        # (the depth of the score-tile ring): right after exp(c) is emitted,
        # emit exactly the PE work that its completion unblocks -- the
        # scores of c+2 (tile WAR) and AV of c -- so the matmul queue head
        # is never parked on a later exp.
        flat = [(key, c) for key in phases for c in range(NCH)]
        sq = []  # pending score tiles, per flat index
        for i, key in enumerate(phases):
            qh, m = key
            st = ensure_state(key)
            if i == 0:
                sq.append(emit_chunk_scores(key, 0))
                sq.append(emit_chunk_scores(key, 1))
            inline_d = key == last_key
            if inline_d:
                Drep_h0 = ps_r.tile([128, 512], F32, tag="rd2")
                Drep_h1 = None
            for c in range(NCH):
                cur_s = sq.pop(0)
                nc.scalar.activation(st["E"][:, c, :], cur_s,
                                     mybir.ActivationFunctionType.Exp,
                                     scale=scale)
                # look ahead 2 chunks (possibly into the next phase)
                fi = i * NCH + c + 2
                if fi < len(flat):
                    nkey, nck = flat[fi]
                    ensure_state(nkey)
                    sq.append(emit_chunk_scores(nkey, nck))
                emit_av(st, c)
                pop_pend(2)
                if c == NCH - 1 and not inline_d:
                    # evacuate outT (bf16, prior-scaled) ahead of the final
                    # tree adds so the accumulator bank rotates sooner
                    ot = npool.tile([128, QHN], BF16, tag="oT", bufs=3)
                    nc.vector.tensor_scalar_mul(ot, st["outT"],
                                                pr_sb[:, m:m + 1])
                    all_res[key][0] = ot
                if c % 2 == 1:
                    j = c // 2
                    nc.vector.tensor_add(st["E2"][:, j, :],
                                         st["E"][:, c - 1, :],
                                         st["E"][:, c, :])
                    if inline_d:
                        sl0, sl1 = slice(0, 512), slice(512, 1024)
                        nc.tensor.matmul(Drep_h0, lhsT=ones_w,
                                         rhs=st["E2"][:, j, sl0],
                                         start=(j == 0), stop=(j == NPAIR - 1))
                        if j >= 2:
                            if j == 2:
                                Drep_h1 = ps_d.tile([128, 512], F32, tag="D")
                            for jj in ([0, 1, 2] if j == 2 else [j]):
                                nc.tensor.matmul(Drep_h1, lhsT=ones_w,
                                                 rhs=st["E2"][:, jj, sl1],
                                                 start=(jj == 0),
                                                 stop=(jj == NPAIR - 1))
                    elif j % 2 == 1:
                        jq = j // 2
                        nc.vector.tensor_add(st["E4"][:, jq, :],
                                             st["E2"][:, j - 1, :],
                                             st["E2"][:, j, :])
            if inline_d:
                ot = npool.tile([128, QHN], BF16, tag="oT", bufs=3)
                nc.vector.tensor_scalar_mul(ot, st["outT"], pr_sb[:, m:m + 1])
                all_res[key][0] = ot
            if inline_d:
                for hf, Drep in ((0, Drep_h0), (1, Drep_h1)):
                    dre = npool.tile([128, 512], F32, tag="drec", bufs=6)
                    nc.vector.reciprocal_approx_fast(dre, Drep)
                    all_res[key][1][hf] = dre
            else:
                if key == (0, 0):
                    # remaining qt staging must precede the denominator
                    # backlog: phase (1,0)'s first scores (emitted near the
                    # end of (0,1)) need qt groups 2-3 transposed+swapped.
                    pend.extend(late_qt)
                pend.extend(make_denom_pend(st["E4"], key))
            if m == M - 1:
                pend.append(lambda qh=qh: emit_combine_half(qh, 0))
                pend.append(lambda qh=qh: emit_combine_half(qh, 1))
            del state[key]
        pop_pend(len(pend))
    return nc


def _get_nc():
    global _NC
    if _NC is None:
        _NC = _build()
        _NC.finalize()  # Bacc.compile(): event sems, reg alloc, wait legalization
    return _NC


def _prior(qt, kernel):
    bar_qt = qt.astype(np.float32).mean(axis=1)          # (BS, dk)
    logits = kernel.astype(np.float32) @ bar_qt.T        # (m, BS)
    z = logits - logits.max(axis=1, keepdims=True)
    ez = np.exp(z)
    pm = ez / ez.sum(axis=1, keepdims=True)              # softmax over batch axis
    return pm.reshape(-1)


def kernel(qt, kt, vt, kernel):
    global LAST_RESULT
    import os
    nc = _get_nc()
    prior_flat = _prior(qt, kernel)
    in_maps = []
    for b in range(BS):
        pr = np.array([[prior_flat[2 * b], prior_flat[2 * b + 1]]], dtype=np.float32)
        in_maps.append({
            "qt_b": np.ascontiguousarray(qt[b], dtype=np.float32),
            "kt_b": np.ascontiguousarray(kt[b], dtype=np.float32),
            "vt_b": np.ascontiguousarray(vt[b], dtype=np.float32),
            "pr_b": pr,
        })
    trace = bool(int(os.environ.get("KERNEL_TRACE", "0")))
    res = run_bass_kernel_spmd(nc, in_maps, list(range(BS)), trace=trace)
    LAST_RESULT = res
    out = np.stack([np.asarray(res.results[b]["out_b"]).reshape(N, DK) for b in range(BS)])
    return out.astype(np.float32)
